# revision 9
# baseline (speedup 1.0000x reference)
"""Trainium2 Bass kernel for a SAGAN-style attention block.

Per batch b:
    xf = x[b].reshape(C, N)                       # C=256, N=4096
    f = (wq / sigma(wq)) @ xf                     # [32, N]
    g = (wk / sigma(wk)) @ xf                     # [32, N]
    h = (wv / sigma(wv)) @ xf                     # [C, N]
    beta = softmax_over_rows(f.T @ g)             # [N, N], softmax over axis 0
    out = gamma * h @ beta + xf

Sharding: 8 cores = (batch b in 0..3) x (column half s in 0..1).  The
softmax normalizes each *column* of the score map over its rows, so a
column shard needs all of f and h but only its own columns of g / the
residual -- shards are fully independent, no cross-core communication.

Per-core kernel layout tricks:
  * scores are built in [n, m] layout (n on partitions) so exp() is a
    plain activation; the softmax denominator is obtained by appending a
    ones-column to h^T so the same accumulating matmul that computes
    (exp(s))^T @ h^T also emits the per-column sum as its 257th output
    column -- and the output lands transposed ([m, c], m on partitions),
    which turns the softmax division into a cheap per-partition
    tensor_scalar multiply.
  * spectral norms (tiny SVDs) + gamma folding are host-side weight prep.
"""

from contextlib import ExitStack

import ml_dtypes
import numpy as np

import concourse.bass as bass
import concourse.tile as tile
from concourse import bacc, mybir
from concourse.bass_utils import run_bass_kernel_spmd

P = 128          # SBUF partitions
C = 256          # value channels
CO = 32          # query/key channels
N = 4096         # H*W sequence length
MS = 2048        # column shard width per core
NCH = N // P     # 32 row chunks of the score map
MTW = 512        # column tile width for the scores matmul
MT = MS // MTW   # 4 column tiles
MSUB = MS // P   # 16 column sub-tiles of 128
F32 = mybir.dt.float32
F32R = mybir.dt.float32r  # fp32 storage, reduced-precision PE mode: 1 cycle/row
BF16 = mybir.dt.bfloat16
# dtype of the exp(s)/hT_aug operands of the big accumulation matmuls.
# bf16 halves their SBUF footprint and lets walrus use the fast weight-load
# path (fp32r weights self-load serially inside the matmul).
ACC_DT = BF16
# dtype of the f/g operands of the scores matmuls (same fast-path reasoning).
SC_DT = BF16
NCORES = 8

_ts = bass.ts


def _mm(nc, out, lhsT, rhs, start, stop):
    """fp32r matmul: 4x faster than fp32 on the PE (one pass instead of two
    half-speed passes); operands carry dtype float32r (fp32 storage)."""
    nc.tensor.matmul(out, lhsT, rhs, start=start, stop=stop)


def _emit(tc: tile.TileContext, xf_d, xres_d, xresT_d, wqT_d, wkT_d, wvT_d, out_d):
    nc = tc.nc
    with ExitStack() as ctx:
        consts = ctx.enter_context(tc.tile_pool(name="consts", bufs=1))

        xf0 = consts.tile([P, N], BF16)
        xf1 = consts.tile([P, N], BF16)
        nc.sync.dma_start(xf0[:], xf_d[0])
        nc.sync.dma_start(xf1[:], xf_d[1])

        xres0 = consts.tile([P, MS], F32R)
        xres1 = consts.tile([P, MS], F32R)
        nc.sync.dma_start(xres0[:], xres_d[0])
        nc.sync.dma_start(xres1[:], xres_d[1])

        xresT_sb = consts.tile([P, MSUB, C], F32)
        for t in range(MSUB):
            nc.sync.dma_start(xresT_sb[:, t, :], xresT_d[t])

        wq0 = consts.tile([P, CO], BF16)
        wq1 = consts.tile([P, CO], BF16)
        wk0 = consts.tile([P, CO], BF16)
        wk1 = consts.tile([P, CO], BF16)
        wv0 = consts.tile([P, C], BF16)
        wv1 = consts.tile([P, C], BF16)
        nc.sync.dma_start(wq0[:], wqT_d[0])
        nc.sync.dma_start(wq1[:], wqT_d[1])
        nc.sync.dma_start(wk0[:], wkT_d[0])
        nc.sync.dma_start(wk1[:], wkT_d[1])
        nc.sync.dma_start(wv0[:], wvT_d[0])
        nc.sync.dma_start(wv1[:], wvT_d[1])

        f_sb = consts.tile([CO, N], SC_DT)
        g_sb = consts.tile([CO, MS], SC_DT)
        # h^T with ones-columns appended per row chunk: [n, c0..c255, 1, 1].
        # Padded to 258 (not 257): fp32r matmul needs an even moving free dim,
        # and memset must go through an f32 view (no f32r memset in the ISA).
        # Column 256 is the softmax denominator; 257 is a dup, ignored.
        hT_sb = consts.tile([P, NCH, C + 2], ACC_DT)
        for k in range(NCH):
            if ACC_DT == F32R:
                nc.vector.memset(hT_sb[:, k, C : C + 2].bitcast(F32), 1.0)
            else:
                nc.vector.memset(hT_sb[:, k, C : C + 2], 1.0)

        with tc.tile_pool(name="pro_ps", bufs=2, space="PSUM") as pro_ps:
            # f = wqn @ xf : [CO, N]
            for t in range(N // MTW):
                ps = pro_ps.tile([CO, MTW], F32, tag="fg")
                _mm(nc, ps[:], wq0[:], xf0[:, _ts(t, MTW)], start=True, stop=False)
                _mm(nc, ps[:], wq1[:], xf1[:, _ts(t, MTW)], start=False, stop=True)
                nc.vector.tensor_copy(f_sb[:, _ts(t, MTW)], ps[:])
            # g = wkn @ xres : [CO, MS]
            for t in range(MT):
                ps = pro_ps.tile([CO, MTW], F32, tag="fg")
                _mm(nc, ps[:], wk0[:], xres0[:, _ts(t, MTW)], start=True, stop=False)
                _mm(nc, ps[:], wk1[:], xres1[:, _ts(t, MTW)], start=False, stop=True)
                nc.vector.tensor_copy(g_sb[:, _ts(t, MTW)], ps[:])
            # hT[n, c] = sum_c' xf[c', n] * wvT[c', c]  (gamma folded into wvT)
            for k in range(NCH):
                ps = pro_ps.tile([P, C], F32, tag="h")
                _mm(nc, ps[:], xf0[:, _ts(k, P)], wv0[:], start=True, stop=False)
                _mm(nc, ps[:], xf1[:, _ts(k, P)], wv1[:], start=False, stop=True)
                nc.vector.tensor_copy(hT_sb[:, k, :C], ps[:])

        with (
            tc.tile_pool(name="sc_ps", bufs=4, space="PSUM") as sc_ps,
            tc.tile_pool(name="acc_ps", bufs=4, space="PSUM") as acc_ps,
            tc.tile_pool(name="work", bufs=4) as work,
            tc.tile_pool(name="outp", bufs=4) as outp,
        ):
            for mt in range(MT):
                accs = [
                    acc_ps.tile([P, C + 2], F32, tag="acc", name=f"acc_{mt}_{sub}")
                    for sub in range(4)
                ]
                for k in range(NCH):
                    # scores[n_chunk, m_tile] = f[:, chunk].T @ g[:, m_tile]
                    sps = sc_ps.tile([P, MTW], F32, tag="s", name=f"s_{mt}_{k}")
                    _mm(nc, sps[:], f_sb[:, _ts(k, P)], g_sb[:, _ts(mt, MTW)],
                        start=True, stop=True)
                    et = work.tile([P, MTW], F32R, tag="e", name=f"e_{mt}_{k}")
                    nc.scalar.activation(et[:], sps[:], mybir.ActivationFunctionType.Exp)
                    # acc[m_sub, c | colsum] += exp(s)[:, sub].T @ hT_aug[chunk]
                    for sub in range(4):
                        _mm(nc, accs[sub][:], et[:, _ts(sub, P)], hT_sb[:, k, :],
                            start=(k == 0), stop=(k == NCH - 1))
                for sub in range(4):
                    mi = mt * 4 + sub
                    rec = work.tile([P, 1], F32, tag="r", name=f"r_{mi}")
                    nc.vector.reciprocal(rec[:], accs[sub][:, C : C + 1])
                    ot = outp.tile([P, C], F32, tag="o", name=f"o_{mi}")
                    nc.vector.tensor_scalar_mul(ot[:], accs[sub][:, :C], rec[:])
                    ot2 = outp.tile([P, C], F32, tag="o2", name=f"o2_{mi}")
                    nc.vector.tensor_add(ot2[:], ot[:], xresT_sb[:, mi, :])
                    nc.sync.dma_start(out_d[mi], ot2[:])


def build_program():
    nc = bacc.Bacc("TRN2", target_bir_lowering=False, debug=False, num_devices=NCORES)
    xf_d = nc.dram_tensor("xf", [2, P, N], BF16, kind="ExternalInput")
    xres_d = nc.dram_tensor("xres", [2, P, MS], F32R, kind="ExternalInput")
    xresT_d = nc.dram_tensor("xresT", [MSUB, P, C], F32, kind="ExternalInput")
    wqT_d = nc.dram_tensor("wqT", [2, P, CO], BF16, kind="ExternalInput")
    wkT_d = nc.dram_tensor("wkT", [2, P, CO], BF16, kind="ExternalInput")
    wvT_d = nc.dram_tensor("wvT", [2, P, C], BF16, kind="ExternalInput")
    out_d = nc.dram_tensor("out", [MSUB, P, C], F32, kind="ExternalOutput")
    with tile.TileContext(nc) as tc:
        _emit(tc, xf_d, xres_d, xresT_d, wqT_d, wkT_d, wvT_d, out_d)
    nc.compile()
    return nc


_PROGRAM = None


def _get_program():
    global _PROGRAM
    if _PROGRAM is None:
        _PROGRAM = build_program()
    return _PROGRAM


def make_in_maps(x, w_q, w_k, w_v, gamma):
    x = np.ascontiguousarray(x, dtype=np.float32)
    wqn = (w_q / np.linalg.norm(w_q, 2)).astype(np.float32)
    wkn = (w_k / np.linalg.norm(w_k, 2)).astype(np.float32)
    wvg = (np.float32(gamma[0]) * (w_v / np.linalg.norm(w_v, 2))).astype(np.float32)
    bf16 = ml_dtypes.bfloat16
    wqT = np.ascontiguousarray(wqn.T).astype(bf16).reshape(2, P, CO)
    wkT = np.ascontiguousarray(wkn.T).astype(bf16).reshape(2, P, CO)
    wvT = np.ascontiguousarray(wvg.T).astype(bf16).reshape(2, P, C)
    B = x.shape[0]
    xf = x.reshape(B, C, N)
    in_maps = []
    for core in range(NCORES):
        b, s = divmod(core, 2)
        xb = xf[b]
        xres = np.ascontiguousarray(xb[:, s * MS : (s + 1) * MS])
        in_maps.append(
            {
                "xf": np.ascontiguousarray(xb).reshape(2, P, N),
                "xres": xres.reshape(2, P, MS),
                "xresT": np.ascontiguousarray(xres.T).reshape(MSUB, P, C),
                "wqT": wqT,
                "wkT": wkT,
                "wvT": wvT,
            }
        )
    return in_maps


def assemble_output(results, x_shape):
    B, _, H, W = x_shape
    out = np.empty((B, C, N), np.float32)
    for core in range(NCORES):
        b, s = divmod(core, 2)
        oT = np.asarray(results[core]["out"]).reshape(MS, C)  # [m, c]
        out[b, :, s * MS : (s + 1) * MS] = oT.T
    return out.reshape(B, C, H, W)


def run(x, w_q, w_k, w_v, gamma, trace=False, **kwargs):
    nc = _get_program()
    in_maps = make_in_maps(x, w_q, w_k, w_v, gamma)
    res = run_bass_kernel_spmd(nc, in_maps, list(range(NCORES)), trace=trace, **kwargs)
    return assemble_output(res.results, x.shape), res


def kernel(x, w_q, w_k, w_v, gamma):
    out, _ = run(
        np.asarray(x), np.asarray(w_q), np.asarray(w_k),
        np.asarray(w_v), np.asarray(gamma),
    )
    return out


# revision 11
# speedup vs baseline: 33.5565x; 33.5565x over previous
"""Trainium2 Bass kernel for a SAGAN-style attention block.

Per batch b:
    xf = x[b].reshape(C, N)                       # C=256, N=4096
    f = (wq / sigma(wq)) @ xf                     # [32, N]
    g = (wk / sigma(wk)) @ xf                     # [32, N]
    h = (wv / sigma(wv)) @ xf                     # [C, N]
    beta = softmax_over_rows(f.T @ g)             # [N, N], softmax over axis 0
    out = gamma * h @ beta + xf

Sharding: 8 cores = (batch b in 0..3) x (column half s in 0..1).  The
softmax normalizes each *column* of the score map over its rows, so a
column shard needs all of f and h but only its own columns of g / the
residual -- shards are fully independent, no cross-core communication.

Per-core kernel layout tricks:
  * scores are built in [n, m] layout (n on partitions) so exp() is a
    plain activation; the softmax denominator is obtained by appending a
    ones-column to h^T so the same accumulating matmul that computes
    (exp(s))^T @ h^T also emits the per-column sum as an extra output
    column -- and the output lands transposed ([m, c], m on partitions),
    which turns the softmax division into a cheap per-partition
    tensor_scalar multiply.
  * matmul operands are bf16 (exp/hT/f/g/xf): full-speed PE streaming and
    the fast weight-load path (fp32 matmul is two half-speed passes, and
    4-byte weights self-load serially inside the matmul).  Accumulation
    stays fp32 in PSUM and the residual add is fp32, keeping the output
    within ~1e-4 relative of the fp32 reference.
  * spectral norms (tiny SVDs) + gamma folding are host-side weight prep.
"""

from contextlib import ExitStack

import ml_dtypes
import numpy as np

import concourse.bass as bass
import concourse.tile as tile
from concourse import bacc, mybir
from concourse.bass_utils import run_bass_kernel_spmd

P = 128          # SBUF partitions
C = 256          # value channels
CO = 32          # query/key channels
N = 4096         # H*W sequence length
MS = 2048        # column shard width per core
NCH = N // P     # 32 row chunks of the score map
MTW = 512        # column tile width for the scores matmul
MT = MS // MTW   # 4 column tiles
MSUB = MS // P   # 16 column sub-tiles of 128
F32 = mybir.dt.float32
F32R = mybir.dt.float32r  # fp32 storage, reduced-precision PE mode: 1 cycle/row
BF16 = mybir.dt.bfloat16
# dtype of the exp(s)/hT_aug operands of the big accumulation matmuls.
# bf16 halves their SBUF footprint and lets walrus use the fast weight-load
# path (fp32r weights self-load serially inside the matmul).
ACC_DT = BF16
# dtype of the f/g operands of the scores matmuls (same fast-path reasoning).
SC_DT = BF16
NCORES = 8

_ts = bass.ts


def _mm(nc, out, lhsT, rhs, start, stop):
    nc.tensor.matmul(out, lhsT, rhs, start=start, stop=stop)


def _emit(tc: tile.TileContext, xf_d, xresT_d, wqT_d, wkT_d, wvT_d, out_d):
    nc = tc.nc
    with ExitStack() as ctx:
        consts = ctx.enter_context(tc.tile_pool(name="consts", bufs=1))

        # DMA order = need order: tiny weights first, then xf in 512-column
        # slices interleaved with the g/f/hT matmuls that chase the stream,
        # and xresT (only needed by the final residual adds) last.
        # Warm the ACT exp table set at t=0 so the ~2.7us PSEUDO_LOAD_ACT
        # table DMA overlaps the input DMA stream instead of delaying the
        # first real exp().
        warm = consts.tile([1, 2], F32)
        nc.vector.memset(warm[:], 0.0)
        nc.scalar.activation(warm[:], warm[:], mybir.ActivationFunctionType.Exp)

        wq0 = consts.tile([P, CO], BF16)
        wq1 = consts.tile([P, CO], BF16)
        wk0 = consts.tile([P, CO], BF16)
        wk1 = consts.tile([P, CO], BF16)
        wv0 = consts.tile([P, C], BF16)
        wv1 = consts.tile([P, C], BF16)
        nc.sync.dma_start(wq0[:], wqT_d[0])
        nc.sync.dma_start(wq1[:], wqT_d[1])
        nc.sync.dma_start(wk0[:], wkT_d[0])
        nc.sync.dma_start(wk1[:], wkT_d[1])
        nc.sync.dma_start(wv0[:], wvT_d[0])
        nc.sync.dma_start(wv1[:], wvT_d[1])

        # xf arrives column-permuted: the core's own m-shard occupies the
        # first MS columns (the host reorders), so g and the residual read
        # xf[:, :MS] uniformly across cores and no separate xres input is
        # needed.  All reductions over n are order-agnostic.
        xf0 = consts.tile([P, N], BF16)
        xf1 = consts.tile([P, N], BF16)

        f_sb = consts.tile([CO, N], SC_DT)
        g_sb = consts.tile([CO, MS], SC_DT)
        # h^T with ones-columns appended per row chunk: [n, c0..c255, 1, 1].
        # Padded to 258 (not 257): the matmul moving free dim must be even
        # (ISA check).  Column 256 becomes the softmax denominator; 257 is a
        # dup, ignored.
        hT_sb = consts.tile([P, NCH, C + 2], ACC_DT)
        for k in range(NCH):
            if ACC_DT == F32R:
                nc.vector.memset(hT_sb[:, k, C : C + 2].bitcast(F32), 1.0)
            else:
                nc.vector.memset(hT_sb[:, k, C : C + 2], 1.0)

        xresT_sb = consts.tile([P, MSUB, C], F32)

        with tc.tile_pool(name="pro_ps", bufs=2, space="PSUM") as pro_ps:
            # Stream xf in 512-col slices; g, f and hT chase the stream.
            for t in range(N // MTW):
                # Split the two c-chunk streams over the HW-DGE and SW-DGE
                # queues so they load in parallel.
                nc.sync.dma_start(xf0[:, _ts(t, MTW)], xf_d[0, :, _ts(t, MTW)])
                nc.gpsimd.dma_start(xf1[:, _ts(t, MTW)], xf_d[1, :, _ts(t, MTW)])
                if t < MT:
                    # g = wkn @ xf[:, :MS] (the core's own columns come first)
                    ps = pro_ps.tile([CO, MTW], F32, tag="fg")
                    _mm(nc, ps[:], wk0[:], xf0[:, _ts(t, MTW)], start=True, stop=False)
                    _mm(nc, ps[:], wk1[:], xf1[:, _ts(t, MTW)], start=False, stop=True)
                    nc.vector.tensor_copy(g_sb[:, _ts(t, MTW)], ps[:])
                # f = wqn @ xf : [CO, 512 slice]
                ps = pro_ps.tile([CO, MTW], F32, tag="fg")
                _mm(nc, ps[:], wq0[:], xf0[:, _ts(t, MTW)], start=True, stop=False)
                _mm(nc, ps[:], wq1[:], xf1[:, _ts(t, MTW)], start=False, stop=True)
                nc.vector.tensor_copy(f_sb[:, _ts(t, MTW)], ps[:])
                # hT[n, c] = sum_c' xf[c', n] * wvT[c', c] (gamma folded in)
                for k in range(4 * t, 4 * t + 4):
                    ps = pro_ps.tile([P, C], F32, tag="h")
                    _mm(nc, ps[:], xf0[:, _ts(k, P)], wv0[:], start=True, stop=False)
                    _mm(nc, ps[:], xf1[:, _ts(k, P)], wv1[:], start=False, stop=True)
                    nc.vector.tensor_copy(hT_sb[:, k, :C], ps[:])
            # Residual (transposed) loads; only needed by the final adds.
            for t in range(MSUB):
                nc.gpsimd.dma_start(xresT_sb[:, t, :], xresT_d[t])

        with (
            tc.tile_pool(name="sc_ps", bufs=2, space="PSUM") as sc_ps,
            tc.tile_pool(name="acc_ps", bufs=4, space="PSUM") as acc_ps,
            tc.tile_pool(name="work", bufs=3) as work,
            tc.tile_pool(name="outp", bufs=4) as outp,
        ):
            for mt in range(MT):
                accs = [
                    acc_ps.tile([P, C + 2], F32, tag="acc", name=f"acc_{mt}_{sub}")
                    for sub in range(4)
                ]

                def emit_accums(kp, et):
                    # acc[m_sub, c | colsum] += exp(s)[:, sub].T @ hT_aug[chunk]
                    for half in range(2):
                        k = 2 * kp + half
                        for sub in range(4):
                            _mm(nc, accs[sub][:],
                                et[:, half * MTW + sub * P : half * MTW + (sub + 1) * P],
                                hT_sb[:, k, :],
                                start=(k == 0), stop=(k == NCH - 1))

                # Software pipeline: emit each pair's accum matmuls two pairs
                # behind its scores+exp, so in PE program order the scores
                # feeding exp(j+2) run before accum(j) -- otherwise the PE
                # finishes both accum batches first and ACT starves waiting
                # for scores (969ns PE bubble per pair in the timeline sim).
                pending = []
                for kp in range(NCH // 2):
                    # Two row chunks share a 2-bank PSUM tile so one exp()
                    # activation covers 1024 elements (less ACT overhead).
                    sps = sc_ps.tile([P, 2 * MTW], F32, tag="s", name=f"s_{mt}_{kp}")
                    et = work.tile([P, 2 * MTW], ACC_DT, tag="e", name=f"e_{mt}_{kp}")
                    for half in range(2):
                        k = 2 * kp + half
                        _mm(nc, sps[:, _ts(half, MTW)], f_sb[:, _ts(k, P)],
                            g_sb[:, _ts(mt, MTW)], start=True, stop=True)
                    nc.scalar.activation(et[:], sps[:], mybir.ActivationFunctionType.Exp)
                    pending.append((kp, et))
                    if len(pending) > 2:
                        emit_accums(*pending.pop(0))
                while pending:
                    emit_accums(*pending.pop(0))
                for sub in range(4):
                    mi = mt * 4 + sub
                    rec = work.tile([P, 1], F32, tag="r", name=f"r_{mi}")
                    nc.vector.reciprocal(rec[:], accs[sub][:, C : C + 1])
                    ot = outp.tile([P, C], F32, tag="o", name=f"o_{mi}")
                    nc.vector.tensor_scalar_mul(ot[:], accs[sub][:, :C], rec[:])
                    ot2 = outp.tile([P, C], F32, tag="o2", name=f"o2_{mi}")
                    nc.vector.tensor_add(ot2[:], ot[:], xresT_sb[:, mi, :])
                    nc.sync.dma_start(out_d[mi], ot2[:])


def build_program(repeat=1):
    nc = bacc.Bacc("TRN2", target_bir_lowering=False, debug=False, num_devices=NCORES)
    xf_d = nc.dram_tensor("xf", [2, P, N], BF16, kind="ExternalInput")
    xresT_d = nc.dram_tensor("xresT", [MSUB, P, C], F32, kind="ExternalInput")
    wqT_d = nc.dram_tensor("wqT", [2, P, CO], BF16, kind="ExternalInput")
    wkT_d = nc.dram_tensor("wkT", [2, P, CO], BF16, kind="ExternalInput")
    wvT_d = nc.dram_tensor("wvT", [2, P, C], BF16, kind="ExternalInput")
    out_d = nc.dram_tensor("out", [MSUB, P, C], F32, kind="ExternalOutput")
    with tile.TileContext(nc) as tc:
        for _ in range(repeat):
            _emit(tc, xf_d, xresT_d, wqT_d, wkT_d, wvT_d, out_d)
    nc.compile()
    return nc


_PROGRAM = None


def _get_program():
    global _PROGRAM
    if _PROGRAM is None:
        _PROGRAM = build_program()
    return _PROGRAM


def make_in_maps(x, w_q, w_k, w_v, gamma):
    x = np.ascontiguousarray(x, dtype=np.float32)
    wqn = (w_q / np.linalg.norm(w_q, 2)).astype(np.float32)
    wkn = (w_k / np.linalg.norm(w_k, 2)).astype(np.float32)
    wvg = (np.float32(gamma[0]) * (w_v / np.linalg.norm(w_v, 2))).astype(np.float32)
    bf16 = ml_dtypes.bfloat16
    wqT = np.ascontiguousarray(wqn.T).astype(bf16).reshape(2, P, CO)
    wkT = np.ascontiguousarray(wkn.T).astype(bf16).reshape(2, P, CO)
    wvT = np.ascontiguousarray(wvg.T).astype(bf16).reshape(2, P, C)
    B = x.shape[0]
    xf = x.reshape(B, C, N)
    in_maps = []
    for core in range(NCORES):
        b, s = divmod(core, 2)
        xb = xf[b]
        xres = np.ascontiguousarray(xb[:, s * MS : (s + 1) * MS])
        other = xb[:, (1 - s) * MS : (2 - s) * MS]
        # Column-permuted xf: own m-shard first (see _emit).
        xperm = np.concatenate([xres, other], axis=1)
        in_maps.append(
            {
                "xf": np.ascontiguousarray(xperm).astype(ml_dtypes.bfloat16).reshape(2, P, N),
                "xresT": np.ascontiguousarray(xres.T).reshape(MSUB, P, C),
                "wqT": wqT,
                "wkT": wkT,
                "wvT": wvT,
            }
        )
    return in_maps


def assemble_output(results, x_shape):
    B, _, H, W = x_shape
    out = np.empty((B, C, N), np.float32)
    for core in range(NCORES):
        b, s = divmod(core, 2)
        oT = np.asarray(results[core]["out"]).reshape(MS, C)  # [m, c]
        out[b, :, s * MS : (s + 1) * MS] = oT.T
    return out.reshape(B, C, H, W)


def run(x, w_q, w_k, w_v, gamma, trace=False, **kwargs):
    nc = _get_program()
    in_maps = make_in_maps(x, w_q, w_k, w_v, gamma)
    res = run_bass_kernel_spmd(nc, in_maps, list(range(NCORES)), trace=trace, **kwargs)
    return assemble_output(res.results, x.shape), res


def kernel(x, w_q, w_k, w_v, gamma):
    out, _ = run(
        np.asarray(x), np.asarray(w_q), np.asarray(w_k),
        np.asarray(w_v), np.asarray(gamma),
    )
    return out



# revision 12
# speedup vs baseline: 52.2175x; 1.5561x over previous
"""Trainium2 Bass kernel for a SAGAN-style attention block.

Per batch b:
    xf = x[b].reshape(C, N)                       # C=256, N=4096
    f = (wq / sigma(wq)) @ xf                     # [32, N]
    g = (wk / sigma(wk)) @ xf                     # [32, N]
    h = (wv / sigma(wv)) @ xf                     # [C, N]
    beta = softmax_over_rows(f.T @ g)             # [N, N], softmax over axis 0
    out = gamma * h @ beta + xf

Sharding: 8 cores = (batch b in 0..3) x (column half s in 0..1).  The
softmax normalizes each *column* of the score map over its rows, so a
column shard needs all of f and h but only its own columns of g / the
residual -- shards are fully independent, no cross-core communication.

Per-core kernel layout tricks:
  * scores are built in [n, m] layout (n on partitions) so exp() is a
    plain activation; the softmax denominator is obtained by appending a
    ones-column to h^T so the same accumulating matmul that computes
    (exp(s))^T @ h^T also emits the per-column sum as an extra output
    column -- and the output lands transposed ([m, c], m on partitions),
    which turns the softmax division into a cheap per-partition
    tensor_scalar multiply.
  * matmul operands are bf16 (exp/hT/f/g/xf): full-speed PE streaming and
    the fast weight-load path (fp32 matmul is two half-speed passes, and
    4-byte weights self-load serially inside the matmul).  Accumulation
    stays fp32 in PSUM and the residual add is fp32, keeping the output
    within ~1e-4 relative of the fp32 reference.
  * spectral norms (tiny SVDs) + gamma folding are host-side weight prep.
"""

from contextlib import ExitStack

import ml_dtypes
import numpy as np

import concourse.bass as bass
import concourse.tile as tile
from concourse import bacc, mybir
from concourse.bass_utils import run_bass_kernel_spmd

P = 128          # SBUF partitions
C = 256          # value channels
CO = 32          # query/key channels
N = 4096         # H*W sequence length
MS = 2048        # column shard width per core
NCH = N // P     # 32 row chunks of the score map
MTW = 512        # column tile width for the scores matmul
MT = MS // MTW   # 4 column tiles
MSUB = MS // P   # 16 column sub-tiles of 128
F32 = mybir.dt.float32
F32R = mybir.dt.float32r  # fp32 storage, reduced-precision PE mode: 1 cycle/row
BF16 = mybir.dt.bfloat16
# dtype of the exp(s)/hT_aug operands of the big accumulation matmuls.
# bf16 halves their SBUF footprint and lets walrus use the fast weight-load
# path (fp32r weights self-load serially inside the matmul).
ACC_DT = BF16
# dtype of the f/g operands of the scores matmuls (same fast-path reasoning).
SC_DT = BF16
NCORES = 8

_ts = bass.ts


def _mm(nc, out, lhsT, rhs, start, stop):
    nc.tensor.matmul(out, lhsT, rhs, start=start, stop=stop)


def _emit(tc: tile.TileContext, xf_d, xresT_d, wqT_d, wkT_d, wvT_d, out_d):
    nc = tc.nc
    with ExitStack() as ctx:
        consts = ctx.enter_context(tc.tile_pool(name="consts", bufs=1))

        # DMA order = need order: tiny weights first, then xf in 512-column
        # slices interleaved with the g/f/hT matmuls that chase the stream,
        # and xresT (only needed by the final residual adds) last.
        # Warm the ACT exp table set at t=0 so the ~2.7us PSEUDO_LOAD_ACT
        # table DMA overlaps the input DMA stream instead of delaying the
        # first real exp().
        warm = consts.tile([1, 2], F32)
        nc.vector.memset(warm[:], 0.0)
        nc.scalar.activation(warm[:], warm[:], mybir.ActivationFunctionType.Exp)

        wq0 = consts.tile([P, CO], BF16)
        wq1 = consts.tile([P, CO], BF16)
        wk0 = consts.tile([P, CO], BF16)
        wk1 = consts.tile([P, CO], BF16)
        wv0 = consts.tile([P, C], BF16)
        wv1 = consts.tile([P, C], BF16)
        nc.sync.dma_start(wq0[:], wqT_d[0])
        nc.sync.dma_start(wq1[:], wqT_d[1])
        nc.sync.dma_start(wk0[:], wkT_d[0])
        nc.sync.dma_start(wk1[:], wkT_d[1])
        nc.sync.dma_start(wv0[:], wvT_d[0])
        nc.sync.dma_start(wv1[:], wvT_d[1])

        # xf arrives column-permuted: the core's own m-shard occupies the
        # first MS columns (the host reorders), so g and the residual read
        # xf[:, :MS] uniformly across cores and no separate xres input is
        # needed.  All reductions over n are order-agnostic.
        xf0 = consts.tile([P, N], BF16)
        xf1 = consts.tile([P, N], BF16)

        f_sb = consts.tile([CO, N], SC_DT)
        g_sb = consts.tile([CO, MS], SC_DT)
        # h^T with ones-columns appended per row chunk: [n, c0..c255, 1, 1].
        # Padded to 258 (not 257): the matmul moving free dim must be even
        # (ISA check).  Column 256 becomes the softmax denominator; 257 is a
        # dup, ignored.
        hT_sb = consts.tile([P, NCH, C + 2], ACC_DT)
        for k in range(NCH):
            if ACC_DT == F32R:
                nc.vector.memset(hT_sb[:, k, C : C + 2].bitcast(F32), 1.0)
            else:
                nc.vector.memset(hT_sb[:, k, C : C + 2], 1.0)

        xresT_sb = consts.tile([P, MSUB, C], F32)

        acc_ps = ctx.enter_context(tc.tile_pool(name="acc_ps", bufs=4, space="PSUM"))
        work = ctx.enter_context(tc.tile_pool(name="work", bufs=3))
        outp = ctx.enter_context(tc.tile_pool(name="outp", bufs=4))

        def final_divide(accs, mt):
            # beta-normalize (per-partition reciprocal of the appended
            # denominator column), add the residual, store.
            for sub in range(4):
                mi = mt * 4 + sub
                rec = work.tile([P, 1], F32, tag="r", name=f"r_{mi}")
                nc.vector.reciprocal(rec[:], accs[sub][:, C : C + 1])
                ot = outp.tile([P, C], F32, tag="o", name=f"o_{mi}")
                nc.vector.tensor_scalar_mul(ot[:], accs[sub][:, :C], rec[:])
                ot2 = outp.tile([P, C], F32, tag="o2", name=f"o2_{mi}")
                nc.vector.tensor_add(ot2[:], ot[:], xresT_sb[:, mi, :])
                nc.sync.dma_start(out_d[mi], ot2[:])

        # ---- Phase A: build f/g/hT chasing the xf DMA stream, with m-tile 0's
        # scores/exp/accum pipeline fused in so the exp stream starts as soon
        # as the first slice lands instead of after the whole prologue.
        # PSUM: pro(2 banks) + 512-wide mt0 scores(2) + acc(4) = 8.
        with (
            tc.tile_pool(name="pro_ps", bufs=2, space="PSUM") as pro_ps,
            tc.tile_pool(name="sc0_ps", bufs=2, space="PSUM") as sc0_ps,
        ):
            accs0 = [
                acc_ps.tile([P, C + 2], F32, tag="acc", name=f"acc_0_{sub}")
                for sub in range(4)
            ]
            pending0 = []

            def emit_accums0(k, et):
                for sub in range(4):
                    _mm(nc, accs0[sub][:], et[:, _ts(sub, P)], hT_sb[:, k, :],
                        start=(k == 0), stop=(k == NCH - 1))

            for t in range(N // MTW):
                # Split the two c-chunk streams over the HW-DGE and SW-DGE
                # queues so they load in parallel.
                nc.sync.dma_start(xf0[:, _ts(t, MTW)], xf_d[0, :, _ts(t, MTW)])
                nc.gpsimd.dma_start(xf1[:, _ts(t, MTW)], xf_d[1, :, _ts(t, MTW)])
                if t < MT:
                    # g = wkn @ xf[:, :MS] (the core's own columns come first)
                    ps = pro_ps.tile([CO, MTW], F32, tag="pro", name=f"gps_{t}")
                    _mm(nc, ps[:], wk0[:], xf0[:, _ts(t, MTW)], start=True, stop=False)
                    _mm(nc, ps[:], wk1[:], xf1[:, _ts(t, MTW)], start=False, stop=True)
                    nc.vector.tensor_copy(g_sb[:, _ts(t, MTW)], ps[:])
                # f = wqn @ xf : [CO, 512 slice]
                ps = pro_ps.tile([CO, MTW], F32, tag="pro", name=f"fps_{t}")
                _mm(nc, ps[:], wq0[:], xf0[:, _ts(t, MTW)], start=True, stop=False)
                _mm(nc, ps[:], wq1[:], xf1[:, _ts(t, MTW)], start=False, stop=True)
                nc.vector.tensor_copy(f_sb[:, _ts(t, MTW)], ps[:])
                for k in range(4 * t, 4 * t + 4):
                    # hT[n, c] = sum_c' xf[c', n] * wvT[c', c] (gamma folded)
                    ps = pro_ps.tile([P, C], F32, tag="pro", name=f"hps_{k}")
                    _mm(nc, ps[:], xf0[:, _ts(k, P)], wv0[:], start=True, stop=False)
                    _mm(nc, ps[:], xf1[:, _ts(k, P)], wv1[:], start=False, stop=True)
                    nc.vector.tensor_copy(hT_sb[:, k, :C], ps[:])
                    # m-tile 0 pipeline chasing the freshly built f/hT chunk
                    sps = sc0_ps.tile([P, MTW], F32, tag="s0", name=f"s0_{k}")
                    _mm(nc, sps[:], f_sb[:, _ts(k, P)], g_sb[:, :MTW],
                        start=True, stop=True)
                    et = work.tile([P, MTW], ACC_DT, tag="e", name=f"e0_{k}")
                    nc.scalar.activation(et[:], sps[:],
                                         mybir.ActivationFunctionType.Exp)
                    pending0.append((k, et))
                    if len(pending0) > 2:
                        emit_accums0(*pending0.pop(0))
            while pending0:
                emit_accums0(*pending0.pop(0))
            # Residual (transposed) loads; only needed by the final adds.
            for t in range(MSUB):
                nc.gpsimd.dma_start(xresT_sb[:, t, :], xresT_d[t])
            final_divide(accs0, 0)

        # ---- Phase B: m-tiles 1..3 with 1024-wide score tiles (pro pools
        # released above: scores 2x2 banks + acc 4 = 8).
        with tc.tile_pool(name="sc_ps", bufs=2, space="PSUM") as sc_ps:
            for mt in range(1, MT):
                accs = [
                    acc_ps.tile([P, C + 2], F32, tag="acc", name=f"acc_{mt}_{sub}")
                    for sub in range(4)
                ]

                def emit_accums(kp, et):
                    # acc[m_sub, c | colsum] += exp(s)[:, sub].T @ hT_aug[chunk]
                    for half in range(2):
                        k = 2 * kp + half
                        for sub in range(4):
                            _mm(nc, accs[sub][:],
                                et[:, half * MTW + sub * P : half * MTW + (sub + 1) * P],
                                hT_sb[:, k, :],
                                start=(k == 0), stop=(k == NCH - 1))

                # Software pipeline: emit each pair's accum matmuls two pairs
                # behind its scores+exp, so in PE program order the scores
                # feeding exp(j+2) run before accum(j) -- otherwise the PE
                # finishes both accum batches first and ACT starves waiting
                # for scores (969ns PE bubble per pair in the timeline sim).
                pending = []
                for kp in range(NCH // 2):
                    # Two row chunks share a 2-bank PSUM tile so one exp()
                    # activation covers 1024 elements (less ACT overhead).
                    sps = sc_ps.tile([P, 2 * MTW], F32, tag="s", name=f"s_{mt}_{kp}")
                    et = work.tile([P, 2 * MTW], ACC_DT, tag="e", name=f"e_{mt}_{kp}")
                    for half in range(2):
                        k = 2 * kp + half
                        _mm(nc, sps[:, _ts(half, MTW)], f_sb[:, _ts(k, P)],
                            g_sb[:, _ts(mt, MTW)], start=True, stop=True)
                    nc.scalar.activation(et[:], sps[:], mybir.ActivationFunctionType.Exp)
                    pending.append((kp, et))
                    if len(pending) > 2:
                        emit_accums(*pending.pop(0))
                while pending:
                    emit_accums(*pending.pop(0))
                final_divide(accs, mt)


def build_program(repeat=1):
    nc = bacc.Bacc("TRN2", target_bir_lowering=False, debug=False, num_devices=NCORES)
    xf_d = nc.dram_tensor("xf", [2, P, N], BF16, kind="ExternalInput")
    xresT_d = nc.dram_tensor("xresT", [MSUB, P, C], F32, kind="ExternalInput")
    wqT_d = nc.dram_tensor("wqT", [2, P, CO], BF16, kind="ExternalInput")
    wkT_d = nc.dram_tensor("wkT", [2, P, CO], BF16, kind="ExternalInput")
    wvT_d = nc.dram_tensor("wvT", [2, P, C], BF16, kind="ExternalInput")
    out_d = nc.dram_tensor("out", [MSUB, P, C], F32, kind="ExternalOutput")
    with tile.TileContext(nc) as tc:
        for _ in range(repeat):
            _emit(tc, xf_d, xresT_d, wqT_d, wkT_d, wvT_d, out_d)
    nc.compile()
    return nc


_PROGRAM = None


def _get_program():
    global _PROGRAM
    if _PROGRAM is None:
        _PROGRAM = build_program()
    return _PROGRAM


def make_in_maps(x, w_q, w_k, w_v, gamma):
    x = np.ascontiguousarray(x, dtype=np.float32)
    wqn = (w_q / np.linalg.norm(w_q, 2)).astype(np.float32)
    wkn = (w_k / np.linalg.norm(w_k, 2)).astype(np.float32)
    wvg = (np.float32(gamma[0]) * (w_v / np.linalg.norm(w_v, 2))).astype(np.float32)
    bf16 = ml_dtypes.bfloat16
    wqT = np.ascontiguousarray(wqn.T).astype(bf16).reshape(2, P, CO)
    wkT = np.ascontiguousarray(wkn.T).astype(bf16).reshape(2, P, CO)
    wvT = np.ascontiguousarray(wvg.T).astype(bf16).reshape(2, P, C)
    B = x.shape[0]
    xf = x.reshape(B, C, N)
    in_maps = []
    for core in range(NCORES):
        b, s = divmod(core, 2)
        xb = xf[b]
        xres = np.ascontiguousarray(xb[:, s * MS : (s + 1) * MS])
        other = xb[:, (1 - s) * MS : (2 - s) * MS]
        # Column-permuted xf: own m-shard first (see _emit).
        xperm = np.concatenate([xres, other], axis=1)
        in_maps.append(
            {
                "xf": np.ascontiguousarray(xperm).astype(ml_dtypes.bfloat16).reshape(2, P, N),
                "xresT": np.ascontiguousarray(xres.T).reshape(MSUB, P, C),
                "wqT": wqT,
                "wkT": wkT,
                "wvT": wvT,
            }
        )
    return in_maps


def assemble_output(results, x_shape):
    B, _, H, W = x_shape
    out = np.empty((B, C, N), np.float32)
    for core in range(NCORES):
        b, s = divmod(core, 2)
        oT = np.asarray(results[core]["out"]).reshape(MS, C)  # [m, c]
        out[b, :, s * MS : (s + 1) * MS] = oT.T
    return out.reshape(B, C, H, W)


def run(x, w_q, w_k, w_v, gamma, trace=False, **kwargs):
    nc = _get_program()
    in_maps = make_in_maps(x, w_q, w_k, w_v, gamma)
    res = run_bass_kernel_spmd(nc, in_maps, list(range(NCORES)), trace=trace, **kwargs)
    return assemble_output(res.results, x.shape), res


def kernel(x, w_q, w_k, w_v, gamma):
    out, _ = run(
        np.asarray(x), np.asarray(w_q), np.asarray(w_k),
        np.asarray(w_v), np.asarray(gamma),
    )
    return out

